# revision 26
# baseline (speedup 1.0000x reference)
"""GCN encoder on 8 Trainium2 NeuronCores (validated: 5.82 ms, rel err 1.1e-7).

Scheme: nodes block-sharded 12500/core; per-core ELL message passing via
dma_gather with int16 indices (4 source windows of 25088 table rows). Each
window gets its own per-core in-degree sort so ELL padding is ~1.16x; window
passes reduce (static strided DVE tensor_reduce) into partial tables, which
a uniform width-4 gather + whole-canvas reduce combines per 14-group chunk.
Normalization is folded (tables pre-scaled by dinv[src], aggregate scaled by
dinv[dst]); self-loops are ordinary edges; mu/logstd share one aggregation.
Layer outputs are AllGathered (8 ranks) into the next layer's gather table.

Perf keys (HW-measured):
- num_swdge_queues=4 with round-robin queue_num on every dma_gather: each
  gather's descriptor generation runs on Q7 core pair (2q, 2q+1), so four
  gathers overlap (Pool exec-queue depth 4) -> ~2.2 ns/row vs 8 ns single
  queue. This is the single biggest win (8.5 ms -> ~6 ms).
- 8 message buffers (x1/x2 chunk tiles rotate in a 2-buf pool to free the
  SBUF) + per-window streamed idx tiles keep enough gathers in flight to
  cover the gen->DMA-land->reduce->buffer-free cycle. CHUNK_COLS=32 is the
  measured sweet spot (48-col chunks and 16-col chunks are both slower).
- L2/L3 reduces process only the live 32 of 64 gathered columns (the
  256-B DMA row minimum forces 64-wide gathers, but the strided DVE
  reduce at ~2.1 ns/elem need not touch the dead half).
- Post-ops (dinv scale, bias, relu) are batched per 14-group chunk via a
  host-precomputed dinv-broadcast tile; next-layer table matmuls are folded
  into the post callbacks so PE work overlaps the Q7 combine tail.
Stability envelope (HW-mapped): single_packet=False, <=4096 idx/gather;
dma_scatter_add avoided (duplicate-dst RMW races); indirect_copy avoided
(wedges the device when >1 instance per program); ap_gather avoided (27
ns/idx, slower than dma_gather).
"""

import numpy as np

N = 100000
NC = 8
NLOC = N // NC
G = 98
NPAD = 128 * G
NFULL = NC * NPAD
WIN = NFULL // 4
F1, F2, F3, F4 = 128, 64, 32, 16
MAX_COLS_PER_GATHER = 64
CHUNK_COLS = 32
NCHUNK = 7                      # combine chunks of groups
CH = [14] * 7

_PROFILE = False
_last_exec_ns = None
_TMPDIR = None


def _wrap_idxs(idxs):
    n = len(idxs)
    assert n % 16 == 0
    w = idxs.reshape(n // 16, 16).T.astype(np.int16)
    return np.tile(w, (8, 1))


def _prow(rank):
    return (rank % 128) * G + (rank // 128)


def _preprocess(edge_index):
    src = np.asarray(edge_index[0], dtype=np.int64)
    dst = np.asarray(edge_index[1], dtype=np.int64)
    loop = np.arange(N, dtype=np.int64)
    src = np.concatenate([src, loop])
    dst = np.concatenate([dst, loop])

    deg = np.bincount(dst, minlength=N).astype(np.float64)
    dinv = np.where(deg > 0, 1.0 / np.sqrt(deg), 0.0).astype(np.float32)

    cores = []
    for c in range(NC):
        lo = c * NLOC
        m = (dst >= lo) & (dst < lo + NLOC)
        s_c = src[m]
        d_c = dst[m] - lo
        degloc = np.bincount(d_c, minlength=NLOC)
        order = np.argsort(-degloc, kind="stable")
        rank_of = np.empty(NLOC, np.int64)
        rank_of[order] = np.arange(NLOC)
        cores.append(dict(s=s_c, d=d_c, order=order, rank_of=rank_of))

    row_of_node = np.empty(N, np.int64)
    for c in range(NC):
        rk = cores[c]["rank_of"]
        row_of_node[c * NLOC:(c + 1) * NLOC] = c * NPAD + _prow(rk)

    for c in range(NC):
        cc = cores[c]
        trow = row_of_node[cc["s"]]
        cc["win"] = trow // WIN
        cc["lidx"] = trow % WIN
        # per-window sorts
        cc["order_r"] = []
        cc["rank_r_of"] = []
        cc["deg_r"] = []
        for r in range(4):
            dr = np.bincount(cc["d"][cc["win"] == r], minlength=NLOC)
            o = np.argsort(-dr, kind="stable")
            ro = np.empty(NLOC, np.int64)
            ro[o] = np.arange(NLOC)
            cc["order_r"].append(o)
            cc["rank_r_of"].append(ro)
            cc["deg_r"].append(dr)

    # per-window per-group widths (cross-core max); sorted desc so
    # W_r[g] = max over cores of deg_r[order_r[128*g]]
    Wr = np.zeros((4, G), np.int32)
    for r in range(4):
        for c in range(NC):
            cc = cores[c]
            top = cc["deg_r"][r][cc["order_r"][r][::128][:G]]
            Wr[r] = np.maximum(Wr[r], top)

    zero_local = _prow(NLOC)    # pad-rank row, zero in every table window

    # pass gather index streams
    idx_pass = []               # [core][r] -> [128, 8*sum(Wr[r])]
    for c in range(NC):
        cc = cores[c]
        per_r = []
        for r in range(4):
            m = cc["win"] == r
            d_r = cc["d"][m]
            li_r = cc["lidx"][m]
            rk = cc["rank_r_of"][r][d_r]
            ordk = np.lexsort((li_r, rk))
            rk_s, li_s = rk[ordk], li_r[ordk]
            start = np.searchsorted(rk_s, np.arange(NLOC))
            end = np.searchsorted(rk_s, np.arange(NLOC) + 1)
            parts = []
            for g in range(G):
                w = int(Wr[r][g])
                if w == 0:
                    continue
                seg = np.full((128, w), zero_local, np.int64)
                for p in range(128):
                    rr = 128 * g + p
                    if rr < NLOC:
                        a, b = start[rr], end[rr]
                        if b > a:
                            seg[p, :b - a] = li_s[a:b]
                parts.append(_wrap_idxs(seg.T.reshape(-1)))
            per_r.append(np.concatenate(parts, axis=1) if parts
                         else np.zeros((128, 16), np.int16))
        idx_pass.append(per_r)

    # combine index streams: window pair A=(P0,P1), B=(P2,P3); final order =
    # total-degree ranks. slot i of pair X selects partial of pass 2X+i.
    idx_comb = []               # [core][pair] -> [128, 8*2*G]
    pad_prow = _prow(NLOC)
    for c in range(NC):
        cc = cores[c]
        pair_streams = []
        for pair in range(2):
            parts = []
            for g in range(G):
                seg = np.empty((128, 2), np.int64)
                for i in range(2):
                    r = 2 * pair + i
                    rowv = np.full(128, pad_prow, np.int64)
                    rr = 128 * g + np.arange(128)
                    real = rr < NLOC
                    nodes = cc["order"][rr[real]]
                    rowv[real] = _prow(cc["rank_r_of"][r][nodes])
                    seg[:, i] = rowv + i * NPAD
                parts.append(_wrap_idxs(seg.T.reshape(-1)))
            pair_streams.append(np.concatenate(parts, axis=1))
        idx_comb.append(pair_streams)

    return dinv, cores, Wr, idx_pass, idx_comb


def _build_program(Wr, pass_len):
    import contextlib
    import concourse.bacc as bacc
    import concourse.mybir as mybir
    import concourse.tile as tile
    from concourse import library_config
    from concourse.masks import make_identity

    dt = mybir.dt
    Alu = mybir.AluOpType
    nc = bacc.Bacc("TRN2", target_bir_lowering=False, debug=False,
                   num_devices=NC, num_swdge_queues=4)
    qrr = [0]

    def next_q():
        qrr[0] = (qrr[0] + 1) % 4
        return qrr[0]

    xT = nc.dram_tensor("xT", [128, NPAD], dt.float32, kind="ExternalInput")
    dinv_d = nc.dram_tensor("dinv", [128, G], dt.float32, kind="ExternalInput")
    w1_d = nc.dram_tensor("w1", [F1, F2], dt.float32, kind="ExternalInput")
    w2_d = nc.dram_tensor("w2", [F2, F3], dt.float32, kind="ExternalInput")
    wmu_d = nc.dram_tensor("wmu", [F3, F4], dt.float32, kind="ExternalInput")
    wls_d = nc.dram_tensor("wls", [F3, F4], dt.float32, kind="ExternalInput")
    b1_d = nc.dram_tensor("b1t", [128, F2], dt.float32, kind="ExternalInput")
    b2_d = nc.dram_tensor("b2t", [128, F3], dt.float32, kind="ExternalInput")
    dinvx_d = nc.dram_tensor("dinvx", [128, G, F3], dt.float32,
                             kind="ExternalInput")
    bmu_d = nc.dram_tensor("bmut", [128, F4], dt.float32, kind="ExternalInput")
    bls_d = nc.dram_tensor("blst", [128, F4], dt.float32, kind="ExternalInput")
    idxp_d = [nc.dram_tensor(f"idxp{r}", [128, pass_len[r]], dt.int16,
                             kind="ExternalInput") for r in range(4)]
    idxc_d = [nc.dram_tensor(f"idxc{p}", [128, 16 * G], dt.int16,
                             kind="ExternalInput") for p in range(2)]
    mu_out = nc.dram_tensor("mu", [128, G, F4], dt.float32,
                            kind="ExternalOutput")
    ls_out = nc.dram_tensor("ls", [128, G, F4], dt.float32,
                            kind="ExternalOutput")

    FW = F2

    with tile.TileContext(nc) as tc:
        with contextlib.ExitStack() as ctx:
            dram = ctx.enter_context(
                tc.tile_pool(name="dram", bufs=1, space="DRAM"))
            consts = ctx.enter_context(tc.tile_pool(name="consts", bufs=1))
            psum_mm = ctx.enter_context(
                tc.tile_pool(name="psum_mm", bufs=4, space="PSUM"))
            psum_tr = ctx.enter_context(
                tc.tile_pool(name="psum_tr", bufs=3, space="PSUM"))
            tabp = ctx.enter_context(tc.tile_pool(name="tabp", bufs=1))
            aggp = ctx.enter_context(tc.tile_pool(name="aggp", bufs=1))
            smallp = ctx.enter_context(tc.tile_pool(name="smallp", bufs=6))

            nc.gpsimd.load_library(library_config.mlp)

            def cload(name, dram_t, shape):
                t = consts.tile(shape, dt.float32, name=name)
                nc.sync.dma_start(t[:], dram_t[:])
                return t

            dinv_sb = cload("dinv_sb", dinv_d, [128, G])
            dinvx_sb = cload("dinvx_sb", dinvx_d, [128, G, F3])
            w1_sb = cload("w1_sb", w1_d, [F1, F2])
            w2_sb = cload("w2_sb", w2_d, [F2, F3])
            wmu_sb = cload("wmu_sb", wmu_d, [F3, F4])
            wls_sb = cload("wls_sb", wls_d, [F3, F4])
            b1_sb = cload("b1_sb", b1_d, [128, F2])
            b2_sb = cload("b2_sb", b2_d, [128, F3])
            bmu_sb = cload("bmu_sb", bmu_d, [128, F4])
            bls_sb = cload("bls_sb", bls_d, [128, F4])
            ident = consts.tile([128, 128], dt.float32, name="ident")
            make_identity(nc, ident[:])

            def store_table(tab_sb, name):
                loc = dram.tile([NPAD, FW], dt.float32, name=name)
                nc.sync.dma_start(
                    loc[:].rearrange("(p g) f -> p g f", p=128), tab_sb[:])
                full = dram.tile([NFULL, FW], dt.float32,
                                 addr_space="Shared", name=name + "_full")
                nc.gpsimd.collective_compute(
                    "AllGather", Alu.bypass,
                    replica_groups=[list(range(NC))],
                    ins=[loc.opt()], outs=[full.opt()],
                )
                return full

            # ---------- Layer 1 matmul ----------
            with tc.tile_pool(name="xTp", bufs=1) as xp:
                xT_sb = xp.tile([128, NPAD], dt.float32, name="xT_sb")
                nc.sync.dma_start(xT_sb[:], xT[:])
                tab_sb = tabp.tile([128, G, FW], dt.float32, tag="tab",
                                   name="tab1_sb")
                for g in range(G):
                    ps = psum_mm.tile([128, FW], dt.float32, space="PSUM",
                                      tag="mm", name=f"mm1_{g}")
                    nc.tensor.matmul(out=ps[:],
                                     lhsT=xT_sb[:, 128 * g:128 * (g + 1)],
                                     rhs=w1_sb[:], start=True, stop=True)
                    nc.vector.tensor_scalar_mul(
                        tab_sb[:, g, :], ps[:], dinv_sb[:, g:g + 1])
                tab1_full = store_table(tab_sb, "tab1")

            idxp = ctx.enter_context(tc.tile_pool(name="idxp", bufs=1))
            idxs_p = ctx.enter_context(tc.tile_pool(name="idxs", bufs=2))
            msgp = ctx.enter_context(tc.tile_pool(name="msgp", bufs=8))
            combp = ctx.enter_context(tc.tile_pool(name="combp", bufs=2))
            max_plen = max(pass_len)
            idxc_sb = {}
            for p in range(2):
                t = idxp.tile([128, 16 * G], dt.int16, tag=f"idxc{p}",
                              name=f"idxct{p}")
                nc.sync.dma_start(t[:], idxc_d[p][:])
                idxc_sb[p] = t

            def aggregate(tab_full, out_cb, phase, rw):
                # 4 window passes into partial tables
                pairs = []
                for pair in range(2):
                    pab = dram.tile([2 * NPAD, FW], dt.float32,
                                    name=f"pab_{phase}_{pair}")
                    pairs.append(pab)
                for r in range(4):
                    idx_t = idxs_p.tile([128, max_plen], dt.int16,
                                        tag="idxs", name=f"ix_{phase}_{r}")
                    nc.sync.dma_start(idx_t[:, 0:pass_len[r]], idxp_d[r][:])
                    P_sb = tabp.tile([128, G, FW], dt.float32, tag="psb",
                                     name=f"psb_{phase}_{r}")
                    for g0 in range(G):
                        if int(Wr[r][g0]) == 0:
                            nc.vector.memset(P_sb[:, g0, 0:rw], 0.0)
                    # pack whole groups into <=CHUNK_COLS-column chunks
                    chunks = []
                    cur, cols = [], 0
                    for g in range(G):
                        w = int(Wr[r][g])
                        if w == 0:
                            continue
                        assert w <= CHUNK_COLS, (r, g, w)
                        if cols + w > CHUNK_COLS:
                            chunks.append((cur, cols))
                            cur, cols = [], 0
                        cur.append((g, w, cols))
                        cols += w
                    if cur:
                        chunks.append((cur, cols))
                    off = 0
                    for ci, (members, cols) in enumerate(chunks):
                        mt = msgp.tile([128, cols, FW], dt.float32,
                                       tag="msg",
                                       name=f"m_{phase}_{r}_{ci}")
                        nc.gpsimd.dma_gather(
                            mt[:], tab_full[r * WIN:(r + 1) * WIN, :],
                            idx_t[:, off:off + 8 * cols],
                            128 * cols, 128 * cols, FW,
                            single_packet=False, queue_num=next_q(),
                        )
                        off += 8 * cols
                        for (g, w, co) in members:
                            nc.vector.tensor_reduce(
                                P_sb[:, g, 0:rw],
                                mt[:, co:co + w, 0:rw]
                                .rearrange("p w f -> p f w"),
                                axis=mybir.AxisListType.X, op=Alu.add)
                    nc.sync.dma_start(
                        pairs[r // 2][(r % 2) * NPAD:(r % 2 + 1) * NPAD, :]
                        .rearrange("(p g) f -> p g f", p=128),
                        P_sb[:])
                # combine: width-4 gather over the two pair tables
                gl0 = 0
                for ci in range(NCHUNK):
                    ng = CH[ci]
                    outs = []
                    for pair in range(2):
                        mt = combp.tile([128, 2 * ng, FW], dt.float32,
                                        tag="cmb",
                                        name=f"cm_{phase}_{ci}_{pair}")
                        nc.gpsimd.dma_gather(
                            mt[:], pairs[pair][:, :],
                            idxc_sb[pair][:, 16 * gl0:16 * (gl0 + ng)],
                            128 * 2 * ng, 128 * 2 * ng, FW,
                            single_packet=False, queue_num=next_q(),
                        )
                        red = combp.tile([128, ng, FW], dt.float32,
                                         tag="crd",
                                         name=f"cr_{phase}_{ci}_{pair}")
                        nc.vector.tensor_reduce(
                            red[:, :, 0:rw],
                            mt[:].rearrange("p (g two) f -> p g f two",
                                            two=2)[:, :, 0:rw, :],
                            axis=mybir.AxisListType.X, op=Alu.add)
                        outs.append(red)
                    comb = combp.tile([128, ng, FW], dt.float32, tag="cfin",
                                      name=f"cf_{phase}_{ci}")
                    nc.vector.tensor_tensor(comb[:, :, 0:rw],
                                            outs[0][:, :, 0:rw],
                                            outs[1][:, :, 0:rw], op=Alu.add)
                    out_cb(ci, gl0, ng, comb)
                    gl0 += ng

            # ---------- Layer 1 aggregate -> x1 ----------
            xcsp = ctx.enter_context(tc.tile_pool(name="xcsp", bufs=2))

            tab_sb2 = tabp.tile([128, G, FW], dt.float32, tag="tab",
                                name="tab2_sb")
            nc.vector.memset(tab_sb2[:], 0.0)

            def l1_post(ci, gl0, ng, comb):
                nc.vector.tensor_tensor(
                    comb[:, :, 0:F3], comb[:, :, 0:F3],
                    dinvx_sb[:, gl0:gl0 + ng, :], op=Alu.mult)
                nc.vector.tensor_tensor(
                    comb[:, :, F3:F2], comb[:, :, F3:F2],
                    dinvx_sb[:, gl0:gl0 + ng, :], op=Alu.mult)
                nc.vector.tensor_tensor(
                    comb[:], comb[:],
                    b1_sb[:].unsqueeze(1).broadcast_to([128, ng, F2]),
                    op=Alu.add)
                x1_c = xcsp.tile([128, ng, F2], dt.float32, tag="x1c",
                                 name=f"x1c_{ci}")
                nc.vector.tensor_scalar(x1_c[:], comb[:],
                                        0.0, None, Alu.max)
                for gi in range(ng):
                    g = gl0 + gi
                    pt = psum_tr.tile([F2, 128], dt.float32, space="PSUM",
                                      tag="tr", name=f"tr2_{g}")
                    nc.tensor.transpose(pt[:], x1_c[:, gi, :], ident[:])
                    x1t = smallp.tile([F2, 128], dt.float32, tag="x1t",
                                      name=f"x1t_{g}")
                    nc.vector.tensor_copy(x1t[:], pt[:])
                    ps = psum_mm.tile([128, FW], dt.float32, space="PSUM",
                                      tag="mm", name=f"mm2_{g}")
                    nc.tensor.matmul(out=ps[:, 0:F3], lhsT=x1t[:],
                                     rhs=w2_sb[:], start=True, stop=True)
                    nc.vector.tensor_scalar_mul(
                        tab_sb2[:, g, 0:F3], ps[:, 0:F3], dinv_sb[:, g:g + 1])

            aggregate(tab1_full, l1_post, "l1", F2)

            # ---------- Layer 2 ----------
            tab2_full = store_table(tab_sb2, "tab2")


            tab_sb3 = tabp.tile([128, G, FW], dt.float32, tag="tab",
                                name="tab3_sb")
            nc.vector.memset(tab_sb3[:], 0.0)

            def l2_post(ci, gl0, ng, comb):
                nc.vector.tensor_tensor(
                    comb[:, :, 0:F3], comb[:, :, 0:F3],
                    dinvx_sb[:, gl0:gl0 + ng, :], op=Alu.mult)
                nc.vector.tensor_tensor(
                    comb[:, :, 0:F3], comb[:, :, 0:F3],
                    b2_sb[:].unsqueeze(1).broadcast_to([128, ng, F3]),
                    op=Alu.add)
                x2_c = xcsp.tile([128, ng, F3], dt.float32, tag="x2c",
                                 name=f"x2c_{ci}")
                nc.vector.tensor_scalar(x2_c[:], comb[:, :, 0:F3],
                                        0.0, None, Alu.max)
                nc.vector.tensor_tensor(
                    tab_sb3[:, gl0:gl0 + ng, 0:F3], x2_c[:],
                    dinvx_sb[:, gl0:gl0 + ng, :], op=Alu.mult)

            aggregate(tab2_full, l2_post, "l2", F3)

            # ---------- Layer 3 ----------
            tab3_full = store_table(tab_sb3, "tab3")

            outp = ctx.enter_context(tc.tile_pool(name="outp", bufs=2))

            def l3_post(ci, gl0, ng, comb):
                nc.vector.tensor_tensor(
                    comb[:, :, 0:F3], comb[:, :, 0:F3],
                    dinvx_sb[:, gl0:gl0 + ng, :], op=Alu.mult)
                mu_c = outp.tile([128, ng, F4], dt.float32, tag="muc",
                                 name=f"mu_c{ci}")
                ls_c = outp.tile([128, ng, F4], dt.float32, tag="lsc",
                                 name=f"ls_c{ci}")
                for gi in range(ng):
                    g = gl0 + gi
                    red = comb[:, gi, :]
                    pt = psum_tr.tile([F3, 128], dt.float32, space="PSUM",
                                      tag="tr", name=f"tr3_{g}")
                    nc.tensor.transpose(pt[:], red[:, 0:F3], ident[:])
                    zt = smallp.tile([F3, 128], dt.float32, tag="x1t",
                                     name=f"zt_{g}")
                    nc.vector.tensor_copy(zt[:], pt[:])
                    pmu = psum_mm.tile([128, FW], dt.float32, space="PSUM",
                                       tag="mm", name=f"pmu_{g}")
                    nc.tensor.matmul(out=pmu[:, 0:F4], lhsT=zt[:],
                                     rhs=wmu_sb[:], start=True, stop=True)
                    nc.vector.tensor_tensor(mu_c[:, gi, :], pmu[:, 0:F4],
                                            bmu_sb[:], op=Alu.add)
                    pls = psum_mm.tile([128, FW], dt.float32, space="PSUM",
                                       tag="mm", name=f"pls_{g}")
                    nc.tensor.matmul(out=pls[:, 0:F4], lhsT=zt[:],
                                     rhs=wls_sb[:], start=True, stop=True)
                    nc.vector.tensor_tensor(ls_c[:, gi, :], pls[:, 0:F4],
                                            bls_sb[:], op=Alu.add)
                nc.sync.dma_start(mu_out[:, gl0:gl0 + ng, :], mu_c[:])
                nc.sync.dma_start(ls_out[:, gl0:gl0 + ng, :], ls_c[:])

            aggregate(tab3_full, l3_post, "l3", F3)

    nc.compile()
    return nc


def kernel(x, edge_index, W1, b1, W2, b2, Wmu, bmu, Wls, bls):
    global _last_exec_ns
    x = np.asarray(x, np.float32)
    dinv, cores, Wr, idx_pass, idx_comb = _preprocess(edge_index)
    pass_len = [idx_pass[0][r].shape[1] for r in range(4)]

    nc = _build_program(Wr, pass_len)

    def btile(b):
        return np.tile(np.asarray(b, np.float32)[None, :], (128, 1))

    in_maps = []
    for c in range(NC):
        cc = cores[c]
        xT = np.zeros((128, NPAD), np.float32)
        xT[:, cc["rank_of"]] = x[c * NLOC:(c + 1) * NLOC].T

        dv = np.zeros((128, G), np.float32)
        rr = np.arange(128)[:, None] + 128 * np.arange(G)[None, :]
        mreal = rr < NLOC
        dv[mreal] = dinv[c * NLOC + cc["order"][rr[mreal]]]

        im = dict(xT=xT, dinv=dv,
                  dinvx=np.repeat(dv[:, :, None], F3, axis=2),
                  w1=np.asarray(W1, np.float32),
                  w2=np.asarray(W2, np.float32),
                  wmu=np.asarray(Wmu, np.float32),
                  wls=np.asarray(Wls, np.float32),
                  b1t=btile(b1), b2t=btile(b2), bmut=btile(bmu),
                  blst=btile(bls))
        for r in range(4):
            im[f"idxp{r}"] = idx_pass[c][r]
        for p in range(2):
            im[f"idxc{p}"] = idx_comb[c][p]
        in_maps.append(im)

    from concourse.bass_utils import run_bass_kernel_spmd
    res = run_bass_kernel_spmd(nc, in_maps, core_ids=list(range(NC)),
                               trace=_PROFILE, tmpdir=_TMPDIR)
    _last_exec_ns = res.exec_time_ns

    mu = np.empty((N, F4), np.float32)
    ls = np.empty((N, F4), np.float32)
    rr = np.arange(128)[:, None] + 128 * np.arange(G)[None, :]
    mreal = rr < NLOC
    for c in range(NC):
        mo = np.asarray(res.results[c]["mu"]).reshape(128, G, F4)
        lo = np.asarray(res.results[c]["ls"]).reshape(128, G, F4)
        nodes = c * NLOC + cores[c]["order"][rr[mreal]]
        mu[nodes] = mo[mreal]
        ls[nodes] = lo[mreal]
    return mu, ls



# revision 28
# speedup vs baseline: 1.0003x; 1.0003x over previous
"""GCN encoder on 8 Trainium2 NeuronCores (validated: 5.82 ms, rel err 1.1e-7).

Scheme: nodes block-sharded 12500/core; per-core ELL message passing via
dma_gather with int16 indices (4 source windows of 25088 table rows). Each
window gets its own per-core in-degree sort so ELL padding is ~1.16x; window
passes reduce (static strided DVE tensor_reduce) into partial tables, which
a uniform width-4 gather + whole-canvas reduce combines per 14-group chunk.
Normalization is folded (tables pre-scaled by dinv[src], aggregate scaled by
dinv[dst]); self-loops are ordinary edges; mu/logstd share one aggregation.
Layer outputs are AllGathered (8 ranks) into the next layer's gather table.

Perf keys (HW-measured):
- num_swdge_queues=4 with round-robin queue_num on every dma_gather: each
  gather's descriptor generation runs on Q7 core pair (2q, 2q+1), so four
  gathers overlap (Pool exec-queue depth 4) -> ~2.2 ns/row vs 8 ns single
  queue. This is the single biggest win (8.5 ms -> ~6 ms).
- 8 message buffers (x1/x2 chunk tiles rotate in a 2-buf pool to free the
  SBUF) + per-window streamed idx tiles keep enough gathers in flight to
  cover the gen->DMA-land->reduce->buffer-free cycle. CHUNK_COLS=32 is the
  measured sweet spot (48-col chunks and 16-col chunks are both slower).
- L2/L3 reduces process only the live 32 of 64 gathered columns (the
  256-B DMA row minimum forces 64-wide gathers, but the strided DVE
  reduce at ~2.1 ns/elem need not touch the dead half).
- Post-ops (dinv scale, bias, relu) are batched per 14-group chunk via a
  host-precomputed dinv-broadcast tile; next-layer table matmuls are folded
  into the post callbacks so PE work overlaps the Q7 combine tail.
Stability envelope (HW-mapped): single_packet=False, <=4096 idx/gather;
dma_scatter_add avoided (duplicate-dst RMW races); indirect_copy avoided
(wedges the device when >1 instance per program); ap_gather avoided (27
ns/idx, slower than dma_gather).
"""

import numpy as np

N = 100000
NC = 8
NLOC = N // NC
G = 98
NPAD = 128 * G
NFULL = NC * NPAD
WIN = NFULL // 4
F1, F2, F3, F4 = 128, 64, 32, 16
MAX_COLS_PER_GATHER = 64
CHUNK_COLS = 32
NCHUNK = 7                      # combine chunks of groups
CH = [14] * 7

_PROFILE = False
_last_exec_ns = None
_TMPDIR = None


def _wrap_idxs(idxs):
    n = len(idxs)
    assert n % 16 == 0
    w = idxs.reshape(n // 16, 16).T.astype(np.int16)
    return np.tile(w, (8, 1))


def _prow(rank):
    return (rank % 128) * G + (rank // 128)


def _preprocess(edge_index):
    src = np.asarray(edge_index[0], dtype=np.int64)
    dst = np.asarray(edge_index[1], dtype=np.int64)
    loop = np.arange(N, dtype=np.int64)
    src = np.concatenate([src, loop])
    dst = np.concatenate([dst, loop])

    deg = np.bincount(dst, minlength=N).astype(np.float64)
    dinv = np.where(deg > 0, 1.0 / np.sqrt(deg), 0.0).astype(np.float32)

    cores = []
    for c in range(NC):
        lo = c * NLOC
        m = (dst >= lo) & (dst < lo + NLOC)
        s_c = src[m]
        d_c = dst[m] - lo
        degloc = np.bincount(d_c, minlength=NLOC)
        order = np.argsort(-degloc, kind="stable")
        rank_of = np.empty(NLOC, np.int64)
        rank_of[order] = np.arange(NLOC)
        cores.append(dict(s=s_c, d=d_c, order=order, rank_of=rank_of))

    row_of_node = np.empty(N, np.int64)
    for c in range(NC):
        rk = cores[c]["rank_of"]
        row_of_node[c * NLOC:(c + 1) * NLOC] = c * NPAD + _prow(rk)

    for c in range(NC):
        cc = cores[c]
        trow = row_of_node[cc["s"]]
        cc["win"] = trow // WIN
        cc["lidx"] = trow % WIN
        # per-window sorts
        cc["order_r"] = []
        cc["rank_r_of"] = []
        cc["deg_r"] = []
        for r in range(4):
            dr = np.bincount(cc["d"][cc["win"] == r], minlength=NLOC)
            o = np.argsort(-dr, kind="stable")
            ro = np.empty(NLOC, np.int64)
            ro[o] = np.arange(NLOC)
            cc["order_r"].append(o)
            cc["rank_r_of"].append(ro)
            cc["deg_r"].append(dr)

    # per-window per-group widths (cross-core max); sorted desc so
    # W_r[g] = max over cores of deg_r[order_r[128*g]]
    Wr = np.zeros((4, G), np.int32)
    for r in range(4):
        for c in range(NC):
            cc = cores[c]
            top = cc["deg_r"][r][cc["order_r"][r][::128][:G]]
            Wr[r] = np.maximum(Wr[r], top)

    zero_local = _prow(NLOC)    # pad-rank row, zero in every table window

    # pass gather index streams
    idx_pass = []               # [core][r] -> [128, 8*sum(Wr[r])]
    for c in range(NC):
        cc = cores[c]
        per_r = []
        for r in range(4):
            m = cc["win"] == r
            d_r = cc["d"][m]
            li_r = cc["lidx"][m]
            rk = cc["rank_r_of"][r][d_r]
            ordk = np.lexsort((li_r, rk))
            rk_s, li_s = rk[ordk], li_r[ordk]
            start = np.searchsorted(rk_s, np.arange(NLOC))
            end = np.searchsorted(rk_s, np.arange(NLOC) + 1)
            parts = []
            for g in range(G):
                w = int(Wr[r][g])
                if w == 0:
                    continue
                seg = np.full((128, w), zero_local, np.int64)
                for p in range(128):
                    rr = 128 * g + p
                    if rr < NLOC:
                        a, b = start[rr], end[rr]
                        if b > a:
                            seg[p, :b - a] = li_s[a:b]
                parts.append(_wrap_idxs(seg.T.reshape(-1)))
            per_r.append(np.concatenate(parts, axis=1) if parts
                         else np.zeros((128, 16), np.int16))
        idx_pass.append(per_r)

    # combine index streams: window pair A=(P0,P1), B=(P2,P3); final order =
    # total-degree ranks. slot i of pair X selects partial of pass 2X+i.
    idx_comb = []               # [core][pair] -> [128, 8*2*G]
    pad_prow = _prow(NLOC)
    for c in range(NC):
        cc = cores[c]
        pair_streams = []
        for pair in range(2):
            parts = []
            for g in range(G):
                seg = np.empty((128, 2), np.int64)
                for i in range(2):
                    r = 2 * pair + i
                    rowv = np.full(128, pad_prow, np.int64)
                    rr = 128 * g + np.arange(128)
                    real = rr < NLOC
                    nodes = cc["order"][rr[real]]
                    rowv[real] = _prow(cc["rank_r_of"][r][nodes])
                    seg[:, i] = rowv + i * NPAD
                parts.append(_wrap_idxs(seg.T.reshape(-1)))
            pair_streams.append(np.concatenate(parts, axis=1))
        idx_comb.append(pair_streams)

    return dinv, cores, Wr, idx_pass, idx_comb


def _build_program(Wr, pass_len):
    import contextlib
    import concourse.bacc as bacc
    import concourse.mybir as mybir
    import concourse.tile as tile
    from concourse import library_config
    from concourse.masks import make_identity

    dt = mybir.dt
    Alu = mybir.AluOpType
    nc = bacc.Bacc("TRN2", target_bir_lowering=False, debug=False,
                   num_devices=NC, num_swdge_queues=4)
    qrr = [0]

    def next_q():
        qrr[0] = (qrr[0] + 1) % 4
        return qrr[0]

    xT = nc.dram_tensor("xT", [128, NPAD], dt.float32, kind="ExternalInput")
    dinv_d = nc.dram_tensor("dinv", [128, G], dt.float32, kind="ExternalInput")
    w1_d = nc.dram_tensor("w1", [F1, F2], dt.float32, kind="ExternalInput")
    w2_d = nc.dram_tensor("w2", [F2, F3], dt.float32, kind="ExternalInput")
    wmu_d = nc.dram_tensor("wmu", [F3, F4], dt.float32, kind="ExternalInput")
    wls_d = nc.dram_tensor("wls", [F3, F4], dt.float32, kind="ExternalInput")
    b1_d = nc.dram_tensor("b1t", [128, F2], dt.float32, kind="ExternalInput")
    b2_d = nc.dram_tensor("b2t", [128, F3], dt.float32, kind="ExternalInput")
    dinvx_d = nc.dram_tensor("dinvx", [128, G, F3], dt.float32,
                             kind="ExternalInput")
    bmu_d = nc.dram_tensor("bmut", [128, F4], dt.float32, kind="ExternalInput")
    bls_d = nc.dram_tensor("blst", [128, F4], dt.float32, kind="ExternalInput")
    idxp_d = [nc.dram_tensor(f"idxp{r}", [128, pass_len[r]], dt.int16,
                             kind="ExternalInput") for r in range(4)]
    idxc_d = [nc.dram_tensor(f"idxc{p}", [128, 16 * G], dt.int16,
                             kind="ExternalInput") for p in range(2)]
    mu_out = nc.dram_tensor("mu", [128, G, F4], dt.float32,
                            kind="ExternalOutput")
    ls_out = nc.dram_tensor("ls", [128, G, F4], dt.float32,
                            kind="ExternalOutput")

    FW = F2

    with tile.TileContext(nc) as tc:
        with contextlib.ExitStack() as ctx:
            dram = ctx.enter_context(
                tc.tile_pool(name="dram", bufs=1, space="DRAM"))
            consts = ctx.enter_context(tc.tile_pool(name="consts", bufs=1))
            psum_mm = ctx.enter_context(
                tc.tile_pool(name="psum_mm", bufs=5, space="PSUM"))
            psum_tr = ctx.enter_context(
                tc.tile_pool(name="psum_tr", bufs=3, space="PSUM"))
            tabp = ctx.enter_context(tc.tile_pool(name="tabp", bufs=1))
            aggp = ctx.enter_context(tc.tile_pool(name="aggp", bufs=1))
            smallp = ctx.enter_context(tc.tile_pool(name="smallp", bufs=8))

            nc.gpsimd.load_library(library_config.mlp)

            def cload(name, dram_t, shape):
                t = consts.tile(shape, dt.float32, name=name)
                nc.sync.dma_start(t[:], dram_t[:])
                return t

            dinv_sb = cload("dinv_sb", dinv_d, [128, G])
            dinvx_sb = cload("dinvx_sb", dinvx_d, [128, G, F3])
            w1_sb = cload("w1_sb", w1_d, [F1, F2])
            w2_sb = cload("w2_sb", w2_d, [F2, F3])
            wmu_sb = cload("wmu_sb", wmu_d, [F3, F4])
            wls_sb = cload("wls_sb", wls_d, [F3, F4])
            b1_sb = cload("b1_sb", b1_d, [128, F2])
            b2_sb = cload("b2_sb", b2_d, [128, F3])
            bmu_sb = cload("bmu_sb", bmu_d, [128, F4])
            bls_sb = cload("bls_sb", bls_d, [128, F4])
            ident = consts.tile([128, 128], dt.float32, name="ident")
            make_identity(nc, ident[:])

            def store_table(tab_sb, name):
                loc = dram.tile([NPAD, FW], dt.float32, name=name)
                nc.sync.dma_start(
                    loc[:].rearrange("(p g) f -> p g f", p=128), tab_sb[:])
                full = dram.tile([NFULL, FW], dt.float32,
                                 addr_space="Shared", name=name + "_full")
                nc.gpsimd.collective_compute(
                    "AllGather", Alu.bypass,
                    replica_groups=[list(range(NC))],
                    ins=[loc.opt()], outs=[full.opt()],
                )
                return full

            # ---------- Layer 1 matmul ----------
            with tc.tile_pool(name="xTp", bufs=1) as xp:
                xT_sb = xp.tile([128, NPAD], dt.float32, name="xT_sb")
                nc.sync.dma_start(xT_sb[:], xT[:])
                tab_sb = tabp.tile([128, G, FW], dt.float32, tag="tab",
                                   name="tab1_sb")
                for g in range(G):
                    ps = psum_mm.tile([128, FW], dt.float32, space="PSUM",
                                      tag="mm", name=f"mm1_{g}")
                    nc.tensor.matmul(out=ps[:],
                                     lhsT=xT_sb[:, 128 * g:128 * (g + 1)],
                                     rhs=w1_sb[:], start=True, stop=True)
                    nc.vector.tensor_scalar_mul(
                        tab_sb[:, g, :], ps[:], dinv_sb[:, g:g + 1])
                tab1_full = store_table(tab_sb, "tab1")

            idxp = ctx.enter_context(tc.tile_pool(name="idxp", bufs=1))
            idxs_p = ctx.enter_context(tc.tile_pool(name="idxs", bufs=2))
            msgp = ctx.enter_context(tc.tile_pool(name="msgp", bufs=9))
            combp = ctx.enter_context(tc.tile_pool(name="combp", bufs=2))
            max_plen = max(pass_len)
            idxc_sb = {}
            for p in range(2):
                t = idxp.tile([128, 16 * G], dt.int16, tag=f"idxc{p}",
                              name=f"idxct{p}")
                nc.sync.dma_start(t[:], idxc_d[p][:])
                idxc_sb[p] = t

            def aggregate(tab_full, out_cb, phase, rw):
                # 4 window passes into partial tables
                pairs = []
                for pair in range(2):
                    pab = dram.tile([2 * NPAD, FW], dt.float32,
                                    name=f"pab_{phase}_{pair}")
                    pairs.append(pab)
                for r in range(4):
                    idx_t = idxs_p.tile([128, max_plen], dt.int16,
                                        tag="idxs", name=f"ix_{phase}_{r}")
                    nc.sync.dma_start(idx_t[:, 0:pass_len[r]], idxp_d[r][:])
                    P_sb = tabp.tile([128, G, FW], dt.float32, tag="psb",
                                     name=f"psb_{phase}_{r}")
                    for g0 in range(G):
                        if int(Wr[r][g0]) == 0:
                            nc.vector.memset(P_sb[:, g0, 0:rw], 0.0)
                    # pack whole groups into <=CHUNK_COLS-column chunks
                    chunks = []
                    cur, cols = [], 0
                    for g in range(G):
                        w = int(Wr[r][g])
                        if w == 0:
                            continue
                        assert w <= CHUNK_COLS, (r, g, w)
                        if cols + w > CHUNK_COLS:
                            chunks.append((cur, cols))
                            cur, cols = [], 0
                        cur.append((g, w, cols))
                        cols += w
                    if cur:
                        chunks.append((cur, cols))
                    off = 0
                    for ci, (members, cols) in enumerate(chunks):
                        mt = msgp.tile([128, cols, FW], dt.float32,
                                       tag="msg",
                                       name=f"m_{phase}_{r}_{ci}")
                        nc.gpsimd.dma_gather(
                            mt[:], tab_full[r * WIN:(r + 1) * WIN, :],
                            idx_t[:, off:off + 8 * cols],
                            128 * cols, 128 * cols, FW,
                            single_packet=False, queue_num=next_q(),
                        )
                        off += 8 * cols
                        for (g, w, co) in members:
                            nc.vector.tensor_reduce(
                                P_sb[:, g, 0:rw],
                                mt[:, co:co + w, 0:rw]
                                .rearrange("p w f -> p f w"),
                                axis=mybir.AxisListType.X, op=Alu.add)
                    nc.sync.dma_start(
                        pairs[r // 2][(r % 2) * NPAD:(r % 2 + 1) * NPAD, :]
                        .rearrange("(p g) f -> p g f", p=128),
                        P_sb[:])
                # combine: width-4 gather over the two pair tables
                gl0 = 0
                for ci in range(NCHUNK):
                    ng = CH[ci]
                    outs = []
                    for pair in range(2):
                        mt = combp.tile([128, 2 * ng, FW], dt.float32,
                                        tag="cmb",
                                        name=f"cm_{phase}_{ci}_{pair}")
                        nc.gpsimd.dma_gather(
                            mt[:], pairs[pair][:, :],
                            idxc_sb[pair][:, 16 * gl0:16 * (gl0 + ng)],
                            128 * 2 * ng, 128 * 2 * ng, FW,
                            single_packet=False, queue_num=next_q(),
                        )
                        red = combp.tile([128, ng, FW], dt.float32,
                                         tag="crd",
                                         name=f"cr_{phase}_{ci}_{pair}")
                        nc.vector.tensor_reduce(
                            red[:, :, 0:rw],
                            mt[:].rearrange("p (g two) f -> p g f two",
                                            two=2)[:, :, 0:rw, :],
                            axis=mybir.AxisListType.X, op=Alu.add)
                        outs.append(red)
                    comb = combp.tile([128, ng, FW], dt.float32, tag="cfin",
                                      name=f"cf_{phase}_{ci}")
                    nc.vector.tensor_tensor(comb[:, :, 0:rw],
                                            outs[0][:, :, 0:rw],
                                            outs[1][:, :, 0:rw], op=Alu.add)
                    out_cb(ci, gl0, ng, comb)
                    gl0 += ng

            # ---------- Layer 1 aggregate -> x1 ----------
            xcsp = ctx.enter_context(tc.tile_pool(name="xcsp", bufs=2))

            tab_sb2 = tabp.tile([128, G, FW], dt.float32, tag="tab",
                                name="tab2_sb")
            nc.vector.memset(tab_sb2[:], 0.0)

            def l1_post(ci, gl0, ng, comb):
                nc.vector.tensor_tensor(
                    comb[:, :, 0:F3], comb[:, :, 0:F3],
                    dinvx_sb[:, gl0:gl0 + ng, :], op=Alu.mult)
                nc.vector.tensor_tensor(
                    comb[:, :, F3:F2], comb[:, :, F3:F2],
                    dinvx_sb[:, gl0:gl0 + ng, :], op=Alu.mult)
                nc.vector.tensor_tensor(
                    comb[:], comb[:],
                    b1_sb[:].unsqueeze(1).broadcast_to([128, ng, F2]),
                    op=Alu.add)
                x1_c = xcsp.tile([128, ng, F2], dt.float32, tag="x1c",
                                 name=f"x1c_{ci}")
                nc.vector.tensor_scalar(x1_c[:], comb[:],
                                        0.0, None, Alu.max)
                for gi in range(ng):
                    g = gl0 + gi
                    pt = psum_tr.tile([F2, 128], dt.float32, space="PSUM",
                                      tag="tr", name=f"tr2_{g}")
                    nc.tensor.transpose(pt[:], x1_c[:, gi, :], ident[:])
                    x1t = smallp.tile([F2, 128], dt.float32, tag="x1t",
                                      name=f"x1t_{g}")
                    nc.vector.tensor_copy(x1t[:], pt[:])
                    ps = psum_mm.tile([128, FW], dt.float32, space="PSUM",
                                      tag="mm", name=f"mm2_{g}")
                    nc.tensor.matmul(out=ps[:, 0:F3], lhsT=x1t[:],
                                     rhs=w2_sb[:], start=True, stop=True)
                    nc.vector.tensor_scalar_mul(
                        tab_sb2[:, g, 0:F3], ps[:, 0:F3], dinv_sb[:, g:g + 1])

            aggregate(tab1_full, l1_post, "l1", F2)

            # ---------- Layer 2 ----------
            tab2_full = store_table(tab_sb2, "tab2")


            tab_sb3 = tabp.tile([128, G, FW], dt.float32, tag="tab",
                                name="tab3_sb")
            nc.vector.memset(tab_sb3[:], 0.0)

            def l2_post(ci, gl0, ng, comb):
                nc.vector.tensor_tensor(
                    comb[:, :, 0:F3], comb[:, :, 0:F3],
                    dinvx_sb[:, gl0:gl0 + ng, :], op=Alu.mult)
                nc.vector.tensor_tensor(
                    comb[:, :, 0:F3], comb[:, :, 0:F3],
                    b2_sb[:].unsqueeze(1).broadcast_to([128, ng, F3]),
                    op=Alu.add)
                x2_c = xcsp.tile([128, ng, F3], dt.float32, tag="x2c",
                                 name=f"x2c_{ci}")
                nc.vector.tensor_scalar(x2_c[:], comb[:, :, 0:F3],
                                        0.0, None, Alu.max)
                nc.vector.tensor_tensor(
                    tab_sb3[:, gl0:gl0 + ng, 0:F3], x2_c[:],
                    dinvx_sb[:, gl0:gl0 + ng, :], op=Alu.mult)

            aggregate(tab2_full, l2_post, "l2", F3)

            # ---------- Layer 3 ----------
            tab3_full = store_table(tab_sb3, "tab3")

            outp = ctx.enter_context(tc.tile_pool(name="outp", bufs=2))

            def l3_post(ci, gl0, ng, comb):
                nc.vector.tensor_tensor(
                    comb[:, :, 0:F3], comb[:, :, 0:F3],
                    dinvx_sb[:, gl0:gl0 + ng, :], op=Alu.mult)
                mu_c = outp.tile([128, ng, F4], dt.float32, tag="muc",
                                 name=f"mu_c{ci}")
                ls_c = outp.tile([128, ng, F4], dt.float32, tag="lsc",
                                 name=f"ls_c{ci}")
                for gi in range(ng):
                    g = gl0 + gi
                    red = comb[:, gi, :]
                    pt = psum_tr.tile([F3, 128], dt.float32, space="PSUM",
                                      tag="tr", name=f"tr3_{g}")
                    nc.tensor.transpose(pt[:], red[:, 0:F3], ident[:])
                    zt = smallp.tile([F3, 128], dt.float32, tag="x1t",
                                     name=f"zt_{g}")
                    nc.vector.tensor_copy(zt[:], pt[:])
                    pmu = psum_mm.tile([128, FW], dt.float32, space="PSUM",
                                       tag="mm", name=f"pmu_{g}")
                    nc.tensor.matmul(out=pmu[:, 0:F4], lhsT=zt[:],
                                     rhs=wmu_sb[:], start=True, stop=True)
                    nc.vector.tensor_tensor(mu_c[:, gi, :], pmu[:, 0:F4],
                                            bmu_sb[:], op=Alu.add)
                    pls = psum_mm.tile([128, FW], dt.float32, space="PSUM",
                                       tag="mm", name=f"pls_{g}")
                    nc.tensor.matmul(out=pls[:, 0:F4], lhsT=zt[:],
                                     rhs=wls_sb[:], start=True, stop=True)
                    nc.vector.tensor_tensor(ls_c[:, gi, :], pls[:, 0:F4],
                                            bls_sb[:], op=Alu.add)
                nc.sync.dma_start(mu_out[:, gl0:gl0 + ng, :], mu_c[:])
                nc.sync.dma_start(ls_out[:, gl0:gl0 + ng, :], ls_c[:])

            aggregate(tab3_full, l3_post, "l3", F3)

    nc.compile()
    return nc


def kernel(x, edge_index, W1, b1, W2, b2, Wmu, bmu, Wls, bls):
    global _last_exec_ns
    x = np.asarray(x, np.float32)
    dinv, cores, Wr, idx_pass, idx_comb = _preprocess(edge_index)
    pass_len = [idx_pass[0][r].shape[1] for r in range(4)]

    nc = _build_program(Wr, pass_len)

    def btile(b):
        return np.tile(np.asarray(b, np.float32)[None, :], (128, 1))

    in_maps = []
    for c in range(NC):
        cc = cores[c]
        xT = np.zeros((128, NPAD), np.float32)
        xT[:, cc["rank_of"]] = x[c * NLOC:(c + 1) * NLOC].T

        dv = np.zeros((128, G), np.float32)
        rr = np.arange(128)[:, None] + 128 * np.arange(G)[None, :]
        mreal = rr < NLOC
        dv[mreal] = dinv[c * NLOC + cc["order"][rr[mreal]]]

        im = dict(xT=xT, dinv=dv,
                  dinvx=np.repeat(dv[:, :, None], F3, axis=2),
                  w1=np.asarray(W1, np.float32),
                  w2=np.asarray(W2, np.float32),
                  wmu=np.asarray(Wmu, np.float32),
                  wls=np.asarray(Wls, np.float32),
                  b1t=btile(b1), b2t=btile(b2), bmut=btile(bmu),
                  blst=btile(bls))
        for r in range(4):
            im[f"idxp{r}"] = idx_pass[c][r]
        for p in range(2):
            im[f"idxc{p}"] = idx_comb[c][p]
        in_maps.append(im)

    from concourse.bass_utils import run_bass_kernel_spmd
    res = run_bass_kernel_spmd(nc, in_maps, core_ids=list(range(NC)),
                               trace=_PROFILE, tmpdir=_TMPDIR)
    _last_exec_ns = res.exec_time_ns

    mu = np.empty((N, F4), np.float32)
    ls = np.empty((N, F4), np.float32)
    rr = np.arange(128)[:, None] + 128 * np.arange(G)[None, :]
    mreal = rr < NLOC
    for c in range(NC):
        mo = np.asarray(res.results[c]["mu"]).reshape(128, G, F4)
        lo = np.asarray(res.results[c]["ls"]).reshape(128, G, F4)
        nodes = c * NLOC + cores[c]["order"][rr[mreal]]
        mu[nodes] = mo[mreal]
        ls[nodes] = lo[mreal]
    return mu, ls



# revision 30
# speedup vs baseline: 1.9212x; 1.9206x over previous
"""GCN encoder on 8 Trainium2 NeuronCores (validated: 5.82 ms, rel err 1.1e-7).

Scheme: nodes block-sharded 12500/core; per-core ELL message passing via
dma_gather with int16 indices (4 source windows of 25088 table rows). Each
window gets its own per-core in-degree sort so ELL padding is ~1.16x; window
passes reduce (static strided DVE tensor_reduce) into partial tables, which
a uniform width-4 gather + whole-canvas reduce combines per 14-group chunk.
Normalization is folded (tables pre-scaled by dinv[src], aggregate scaled by
dinv[dst]); self-loops are ordinary edges; mu/logstd share one aggregation.
Layer outputs are AllGathered (8 ranks) into the next layer's gather table.

Perf keys (HW-measured):
- num_swdge_queues=4 with round-robin queue_num on every dma_gather: each
  gather's descriptor generation runs on Q7 core pair (2q, 2q+1), so four
  gathers overlap (Pool exec-queue depth 4) -> ~2.2 ns/row vs 8 ns single
  queue. This is the single biggest win (8.5 ms -> ~6 ms).
- 8 message buffers (x1/x2 chunk tiles rotate in a 2-buf pool to free the
  SBUF) + per-window streamed idx tiles keep enough gathers in flight to
  cover the gen->DMA-land->reduce->buffer-free cycle. CHUNK_COLS=32 is the
  measured sweet spot (48-col chunks and 16-col chunks are both slower).
- L2/L3 reduces process only the live 32 of 64 gathered columns (the
  256-B DMA row minimum forces 64-wide gathers, but the strided DVE
  reduce at ~2.1 ns/elem need not touch the dead half).
- Post-ops (dinv scale, bias, relu) are batched per 14-group chunk via a
  host-precomputed dinv-broadcast tile; next-layer table matmuls are folded
  into the post callbacks so PE work overlaps the Q7 combine tail.
Stability envelope (HW-mapped): single_packet=False, <=4096 idx/gather;
dma_scatter_add avoided (duplicate-dst RMW races); indirect_copy avoided
(wedges the device when >1 instance per program); ap_gather avoided (27
ns/idx, slower than dma_gather).
"""

import numpy as np

N = 100000
NC = 8
NLOC = N // NC
G = 98
NPAD = 128 * G
NFULL = NC * NPAD
WIN = NFULL // 4
F1, F2, F3, F4 = 128, 64, 32, 16
MAX_COLS_PER_GATHER = 64
CHUNK_COLS = 32
NCHUNK = 7                      # combine chunks of groups
CH = [14] * 7

_PROFILE = False
_last_exec_ns = None
_TMPDIR = None


def _wrap_idxs(idxs):
    n = len(idxs)
    assert n % 16 == 0
    w = idxs.reshape(n // 16, 16).T.astype(np.int16)
    return np.tile(w, (8, 1))


def _prow(rank):
    return (rank % 128) * G + (rank // 128)


def _preprocess(edge_index):
    src = np.asarray(edge_index[0], dtype=np.int64)
    dst = np.asarray(edge_index[1], dtype=np.int64)
    loop = np.arange(N, dtype=np.int64)
    src = np.concatenate([src, loop])
    dst = np.concatenate([dst, loop])

    deg = np.bincount(dst, minlength=N).astype(np.float64)
    dinv = np.where(deg > 0, 1.0 / np.sqrt(deg), 0.0).astype(np.float32)

    cores = []
    for c in range(NC):
        lo = c * NLOC
        m = (dst >= lo) & (dst < lo + NLOC)
        s_c = src[m]
        d_c = dst[m] - lo
        degloc = np.bincount(d_c, minlength=NLOC)
        order = np.argsort(-degloc, kind="stable")
        rank_of = np.empty(NLOC, np.int64)
        rank_of[order] = np.arange(NLOC)
        cores.append(dict(s=s_c, d=d_c, order=order, rank_of=rank_of))

    row_of_node = np.empty(N, np.int64)
    for c in range(NC):
        rk = cores[c]["rank_of"]
        row_of_node[c * NLOC:(c + 1) * NLOC] = c * NPAD + _prow(rk)

    B = NPAD // 4          # 3136-row prow band per core per window
    for c in range(NC):
        cc = cores[c]
        trow = row_of_node[cc["s"]]
        prow = trow % NPAD
        cc["win"] = prow // B
        cc["lidx"] = (trow // NPAD) * (B + 1) + prow % B
        # per-window sorts
        cc["order_r"] = []
        cc["rank_r_of"] = []
        cc["deg_r"] = []
        for r in range(4):
            dr = np.bincount(cc["d"][cc["win"] == r], minlength=NLOC)
            o = np.argsort(-dr, kind="stable")
            ro = np.empty(NLOC, np.int64)
            ro[o] = np.arange(NLOC)
            cc["order_r"].append(o)
            cc["rank_r_of"].append(ro)
            cc["deg_r"].append(dr)

    # per-window per-group widths (cross-core max); sorted desc so
    # W_r[g] = max over cores of deg_r[order_r[128*g]]
    Wr = np.zeros((4, G), np.int32)
    for r in range(4):
        for c in range(NC):
            cc = cores[c]
            top = cc["deg_r"][r][cc["order_r"][r][::128][:G]]
            Wr[r] = np.maximum(Wr[r], top)

    zero_local = NPAD // 4      # core-0 zero row in every window tensor

    # pass gather index streams
    idx_pass = []               # [core][r] -> [128, 8*sum(Wr[r])]
    for c in range(NC):
        cc = cores[c]
        per_r = []
        for r in range(4):
            m = cc["win"] == r
            d_r = cc["d"][m]
            li_r = cc["lidx"][m]
            rk = cc["rank_r_of"][r][d_r]
            ordk = np.lexsort((li_r, rk))
            rk_s, li_s = rk[ordk], li_r[ordk]
            start = np.searchsorted(rk_s, np.arange(NLOC))
            end = np.searchsorted(rk_s, np.arange(NLOC) + 1)
            parts = []
            for g in range(G):
                w = int(Wr[r][g])
                if w == 0:
                    continue
                seg = np.full((128, w), zero_local, np.int64)
                for p in range(128):
                    rr = 128 * g + p
                    if rr < NLOC:
                        a, b = start[rr], end[rr]
                        if b > a:
                            seg[p, :b - a] = li_s[a:b]
                parts.append(_wrap_idxs(seg.T.reshape(-1)))
            per_r.append(np.concatenate(parts, axis=1) if parts
                         else np.zeros((128, 16), np.int16))
        idx_pass.append(per_r)

    # combine index streams: window pair A=(P0,P1), B=(P2,P3); final order =
    # total-degree ranks. slot i of pair X selects partial of pass 2X+i.
    idx_comb = []               # [core][pair] -> [128, 8*2*G]
    pad_prow = _prow(NLOC)
    for c in range(NC):
        cc = cores[c]
        pair_streams = []
        for pair in range(2):
            parts = []
            for g in range(G):
                seg = np.empty((128, 2), np.int64)
                for i in range(2):
                    r = 2 * pair + i
                    rowv = np.full(128, pad_prow, np.int64)
                    rr = 128 * g + np.arange(128)
                    real = rr < NLOC
                    nodes = cc["order"][rr[real]]
                    rowv[real] = _prow(cc["rank_r_of"][r][nodes])
                    seg[:, i] = rowv + i * NPAD
                parts.append(_wrap_idxs(seg.T.reshape(-1)))
            pair_streams.append(np.concatenate(parts, axis=1))
        idx_comb.append(pair_streams)

    return dinv, cores, Wr, idx_pass, idx_comb


def _build_program(Wr, pass_len):
    import contextlib
    import concourse.bacc as bacc
    import concourse.mybir as mybir
    import concourse.tile as tile
    from concourse import library_config
    from concourse.masks import make_identity

    dt = mybir.dt
    Alu = mybir.AluOpType
    nc = bacc.Bacc("TRN2", target_bir_lowering=False, debug=False,
                   num_devices=NC, num_swdge_queues=4)
    qrr = [0]

    def next_q():
        qrr[0] = (qrr[0] + 1) % 4
        return qrr[0]

    xT = nc.dram_tensor("xT", [128, NPAD], dt.float32, kind="ExternalInput")
    dinv_d = nc.dram_tensor("dinv", [128, G], dt.float32, kind="ExternalInput")
    w1_d = nc.dram_tensor("w1", [F1, F2], dt.float32, kind="ExternalInput")
    w2_d = nc.dram_tensor("w2", [F2, F3], dt.float32, kind="ExternalInput")
    wmu_d = nc.dram_tensor("wmu", [F3, F4], dt.float32, kind="ExternalInput")
    wls_d = nc.dram_tensor("wls", [F3, F4], dt.float32, kind="ExternalInput")
    b1_d = nc.dram_tensor("b1t", [128, F2], dt.float32, kind="ExternalInput")
    b2_d = nc.dram_tensor("b2t", [128, F3], dt.float32, kind="ExternalInput")
    dinvx_d = nc.dram_tensor("dinvx", [128, G, F3], dt.float32,
                             kind="ExternalInput")
    bmu_d = nc.dram_tensor("bmut", [128, F4], dt.float32, kind="ExternalInput")
    bls_d = nc.dram_tensor("blst", [128, F4], dt.float32, kind="ExternalInput")
    idxp_d = [nc.dram_tensor(f"idxp{r}", [128, pass_len[r]], dt.int16,
                             kind="ExternalInput") for r in range(4)]
    idxc_d = [nc.dram_tensor(f"idxc{p}", [128, 16 * G], dt.int16,
                             kind="ExternalInput") for p in range(2)]
    mu_out = nc.dram_tensor("mu", [128, G, F4], dt.float32,
                            kind="ExternalOutput")
    ls_out = nc.dram_tensor("ls", [128, G, F4], dt.float32,
                            kind="ExternalOutput")

    FW = F2

    with tile.TileContext(nc) as tc:
        with contextlib.ExitStack() as ctx:
            dram = ctx.enter_context(
                tc.tile_pool(name="dram", bufs=1, space="DRAM"))
            consts = ctx.enter_context(tc.tile_pool(name="consts", bufs=1))
            psum_mm = ctx.enter_context(
                tc.tile_pool(name="psum_mm", bufs=5, space="PSUM"))
            psum_tr = ctx.enter_context(
                tc.tile_pool(name="psum_tr", bufs=3, space="PSUM"))
            tabp = ctx.enter_context(tc.tile_pool(name="tabp", bufs=1))
            aggp = ctx.enter_context(tc.tile_pool(name="aggp", bufs=1))
            smallp = ctx.enter_context(tc.tile_pool(name="smallp", bufs=8))

            nc.gpsimd.load_library(library_config.mlp)

            def cload(name, dram_t, shape):
                t = consts.tile(shape, dt.float32, name=name)
                nc.sync.dma_start(t[:], dram_t[:])
                return t

            dinv_sb = cload("dinv_sb", dinv_d, [128, G])
            dinvx_sb = cload("dinvx_sb", dinvx_d, [128, G, F3])
            w1_sb = cload("w1_sb", w1_d, [F1, F2])
            w2_sb = cload("w2_sb", w2_d, [F2, F3])
            wmu_sb = cload("wmu_sb", wmu_d, [F3, F4])
            wls_sb = cload("wls_sb", wls_d, [F3, F4])
            b1_sb = cload("b1_sb", b1_d, [128, F2])
            b2_sb = cload("b2_sb", b2_d, [128, F3])
            bmu_sb = cload("bmu_sb", bmu_d, [128, F4])
            bls_sb = cload("bls_sb", bls_d, [128, F4])
            ident = consts.tile([128, 128], dt.float32, name="ident")
            make_identity(nc, ident[:])

            zrow = consts.tile([1, FW], dt.float32, name="zrow")
            nc.vector.memset(zrow[:], 0.0)
            B = NPAD // 4
            B1 = B + 1

            def store_table(tab_sb, name):
                loc = dram.tile([4 * B1, FW], dt.float32, name=name)
                fulls = []
                for j in range(4):
                    nc.sync.dma_start(
                        loc[j * B1:j * B1 + B, :]
                        .rearrange("(p g) f -> p g f", p=32),
                        tab_sb[32 * j:32 * (j + 1), :, :])
                    nc.sync.dma_start(loc[j * B1 + B:j * B1 + B1, :],
                                      zrow[:])
                    fj = dram.tile([NC * B1, FW], dt.float32,
                                   addr_space="Shared",
                                   name=f"{name}_w{j}")
                    nc.gpsimd.collective_compute(
                        "AllGather", Alu.bypass,
                        replica_groups=[list(range(NC))],
                        ins=[loc[j * B1:(j + 1) * B1, :].opt()],
                        outs=[fj[:].opt()],
                    )
                    fulls.append(fj)
                return fulls

            # ---------- Layer 1 matmul ----------
            with tc.tile_pool(name="xTp", bufs=1) as xp:
                xT_sb = xp.tile([128, NPAD], dt.float32, name="xT_sb")
                nc.sync.dma_start(xT_sb[:], xT[:])
                tab_sb = tabp.tile([128, G, FW], dt.float32, tag="tab",
                                   name="tab1_sb")
                for g in range(G):
                    ps = psum_mm.tile([128, FW], dt.float32, space="PSUM",
                                      tag="mm", name=f"mm1_{g}")
                    nc.tensor.matmul(out=ps[:],
                                     lhsT=xT_sb[:, 128 * g:128 * (g + 1)],
                                     rhs=w1_sb[:], start=True, stop=True)
                    nc.vector.tensor_scalar_mul(
                        tab_sb[:, g, :], ps[:], dinv_sb[:, g:g + 1])
                tab1_full = store_table(tab_sb, "tab1")

            idxp = ctx.enter_context(tc.tile_pool(name="idxp", bufs=1))
            idxs_p = ctx.enter_context(tc.tile_pool(name="idxs", bufs=2))
            msgp = ctx.enter_context(tc.tile_pool(name="msgp", bufs=9))
            combp = ctx.enter_context(tc.tile_pool(name="combp", bufs=2))
            max_plen = max(pass_len)
            idxc_sb = {}
            for p in range(2):
                t = idxp.tile([128, 16 * G], dt.int16, tag=f"idxc{p}",
                              name=f"idxct{p}")
                nc.sync.dma_start(t[:], idxc_d[p][:])
                idxc_sb[p] = t

            def aggregate(tab_full, out_cb, phase, rw):
                # 4 window passes into partial tables
                pairs = []
                for pair in range(2):
                    pab = dram.tile([2 * NPAD, FW], dt.float32,
                                    name=f"pab_{phase}_{pair}")
                    pairs.append(pab)
                for r in range(4):
                    idx_t = idxs_p.tile([128, max_plen], dt.int16,
                                        tag="idxs", name=f"ix_{phase}_{r}")
                    nc.sync.dma_start(idx_t[:, 0:pass_len[r]], idxp_d[r][:])
                    P_sb = tabp.tile([128, G, FW], dt.float32, tag="psb",
                                     name=f"psb_{phase}_{r}")
                    for g0 in range(G):
                        if int(Wr[r][g0]) == 0:
                            nc.vector.memset(P_sb[:, g0, 0:rw], 0.0)
                    # pack whole groups into <=CHUNK_COLS-column chunks
                    chunks = []
                    cur, cols = [], 0
                    for g in range(G):
                        w = int(Wr[r][g])
                        if w == 0:
                            continue
                        assert w <= CHUNK_COLS, (r, g, w)
                        if cols + w > CHUNK_COLS:
                            chunks.append((cur, cols))
                            cur, cols = [], 0
                        cur.append((g, w, cols))
                        cols += w
                    if cur:
                        chunks.append((cur, cols))
                    off = 0
                    for ci, (members, cols) in enumerate(chunks):
                        mt = msgp.tile([128, cols, FW], dt.float32,
                                       tag="msg",
                                       name=f"m_{phase}_{r}_{ci}")
                        nc.gpsimd.dma_gather(
                            mt[:], tab_full[r][:, :],
                            idx_t[:, off:off + 8 * cols],
                            128 * cols, 128 * cols, FW,
                            single_packet=False, queue_num=next_q(),
                        )
                        off += 8 * cols
                        for (g, w, co) in members:
                            nc.vector.tensor_reduce(
                                P_sb[:, g, 0:rw],
                                mt[:, co:co + w, 0:rw]
                                .rearrange("p w f -> p f w"),
                                axis=mybir.AxisListType.X, op=Alu.add)
                    nc.sync.dma_start(
                        pairs[r // 2][(r % 2) * NPAD:(r % 2 + 1) * NPAD, :]
                        .rearrange("(p g) f -> p g f", p=128),
                        P_sb[:])
                # combine: width-4 gather over the two pair tables
                gl0 = 0
                for ci in range(NCHUNK):
                    ng = CH[ci]
                    outs = []
                    for pair in range(2):
                        mt = combp.tile([128, 2 * ng, FW], dt.float32,
                                        tag="cmb",
                                        name=f"cm_{phase}_{ci}_{pair}")
                        nc.gpsimd.dma_gather(
                            mt[:], pairs[pair][:, :],
                            idxc_sb[pair][:, 16 * gl0:16 * (gl0 + ng)],
                            128 * 2 * ng, 128 * 2 * ng, FW,
                            single_packet=False, queue_num=next_q(),
                        )
                        red = combp.tile([128, ng, FW], dt.float32,
                                         tag="crd",
                                         name=f"cr_{phase}_{ci}_{pair}")
                        nc.vector.tensor_reduce(
                            red[:, :, 0:rw],
                            mt[:].rearrange("p (g two) f -> p g f two",
                                            two=2)[:, :, 0:rw, :],
                            axis=mybir.AxisListType.X, op=Alu.add)
                        outs.append(red)
                    comb = combp.tile([128, ng, FW], dt.float32, tag="cfin",
                                      name=f"cf_{phase}_{ci}")
                    nc.vector.tensor_tensor(comb[:, :, 0:rw],
                                            outs[0][:, :, 0:rw],
                                            outs[1][:, :, 0:rw], op=Alu.add)
                    out_cb(ci, gl0, ng, comb)
                    gl0 += ng

            # ---------- Layer 1 aggregate -> x1 ----------
            xcsp = ctx.enter_context(tc.tile_pool(name="xcsp", bufs=2))

            tab_sb2 = tabp.tile([128, G, FW], dt.float32, tag="tab",
                                name="tab2_sb")
            nc.vector.memset(tab_sb2[:], 0.0)

            def l1_post(ci, gl0, ng, comb):
                nc.vector.tensor_tensor(
                    comb[:, :, 0:F3], comb[:, :, 0:F3],
                    dinvx_sb[:, gl0:gl0 + ng, :], op=Alu.mult)
                nc.vector.tensor_tensor(
                    comb[:, :, F3:F2], comb[:, :, F3:F2],
                    dinvx_sb[:, gl0:gl0 + ng, :], op=Alu.mult)
                nc.vector.tensor_tensor(
                    comb[:], comb[:],
                    b1_sb[:].unsqueeze(1).broadcast_to([128, ng, F2]),
                    op=Alu.add)
                x1_c = xcsp.tile([128, ng, F2], dt.float32, tag="x1c",
                                 name=f"x1c_{ci}")
                nc.vector.tensor_scalar(x1_c[:], comb[:],
                                        0.0, None, Alu.max)
                for gi in range(ng):
                    g = gl0 + gi
                    pt = psum_tr.tile([F2, 128], dt.float32, space="PSUM",
                                      tag="tr", name=f"tr2_{g}")
                    nc.tensor.transpose(pt[:], x1_c[:, gi, :], ident[:])
                    x1t = smallp.tile([F2, 128], dt.float32, tag="x1t",
                                      name=f"x1t_{g}")
                    nc.vector.tensor_copy(x1t[:], pt[:])
                    ps = psum_mm.tile([128, FW], dt.float32, space="PSUM",
                                      tag="mm", name=f"mm2_{g}")
                    nc.tensor.matmul(out=ps[:, 0:F3], lhsT=x1t[:],
                                     rhs=w2_sb[:], start=True, stop=True)
                    nc.vector.tensor_scalar_mul(
                        tab_sb2[:, g, 0:F3], ps[:, 0:F3], dinv_sb[:, g:g + 1])

            aggregate(tab1_full, l1_post, "l1", F2)

            # ---------- Layer 2 ----------
            tab2_full = store_table(tab_sb2, "tab2")


            tab_sb3 = tabp.tile([128, G, FW], dt.float32, tag="tab",
                                name="tab3_sb")
            nc.vector.memset(tab_sb3[:], 0.0)

            def l2_post(ci, gl0, ng, comb):
                nc.vector.tensor_tensor(
                    comb[:, :, 0:F3], comb[:, :, 0:F3],
                    dinvx_sb[:, gl0:gl0 + ng, :], op=Alu.mult)
                nc.vector.tensor_tensor(
                    comb[:, :, 0:F3], comb[:, :, 0:F3],
                    b2_sb[:].unsqueeze(1).broadcast_to([128, ng, F3]),
                    op=Alu.add)
                x2_c = xcsp.tile([128, ng, F3], dt.float32, tag="x2c",
                                 name=f"x2c_{ci}")
                nc.vector.tensor_scalar(x2_c[:], comb[:, :, 0:F3],
                                        0.0, None, Alu.max)
                nc.vector.tensor_tensor(
                    tab_sb3[:, gl0:gl0 + ng, 0:F3], x2_c[:],
                    dinvx_sb[:, gl0:gl0 + ng, :], op=Alu.mult)

            aggregate(tab2_full, l2_post, "l2", F3)

            # ---------- Layer 3 ----------
            tab3_full = store_table(tab_sb3, "tab3")

            outp = ctx.enter_context(tc.tile_pool(name="outp", bufs=2))

            def l3_post(ci, gl0, ng, comb):
                nc.vector.tensor_tensor(
                    comb[:, :, 0:F3], comb[:, :, 0:F3],
                    dinvx_sb[:, gl0:gl0 + ng, :], op=Alu.mult)
                mu_c = outp.tile([128, ng, F4], dt.float32, tag="muc",
                                 name=f"mu_c{ci}")
                ls_c = outp.tile([128, ng, F4], dt.float32, tag="lsc",
                                 name=f"ls_c{ci}")
                for gi in range(ng):
                    g = gl0 + gi
                    red = comb[:, gi, :]
                    pt = psum_tr.tile([F3, 128], dt.float32, space="PSUM",
                                      tag="tr", name=f"tr3_{g}")
                    nc.tensor.transpose(pt[:], red[:, 0:F3], ident[:])
                    zt = smallp.tile([F3, 128], dt.float32, tag="x1t",
                                     name=f"zt_{g}")
                    nc.vector.tensor_copy(zt[:], pt[:])
                    pmu = psum_mm.tile([128, FW], dt.float32, space="PSUM",
                                       tag="mm", name=f"pmu_{g}")
                    nc.tensor.matmul(out=pmu[:, 0:F4], lhsT=zt[:],
                                     rhs=wmu_sb[:], start=True, stop=True)
                    nc.vector.tensor_tensor(mu_c[:, gi, :], pmu[:, 0:F4],
                                            bmu_sb[:], op=Alu.add)
                    pls = psum_mm.tile([128, FW], dt.float32, space="PSUM",
                                       tag="mm", name=f"pls_{g}")
                    nc.tensor.matmul(out=pls[:, 0:F4], lhsT=zt[:],
                                     rhs=wls_sb[:], start=True, stop=True)
                    nc.vector.tensor_tensor(ls_c[:, gi, :], pls[:, 0:F4],
                                            bls_sb[:], op=Alu.add)
                nc.sync.dma_start(mu_out[:, gl0:gl0 + ng, :], mu_c[:])
                nc.sync.dma_start(ls_out[:, gl0:gl0 + ng, :], ls_c[:])

            aggregate(tab3_full, l3_post, "l3", F3)

    nc.compile()
    return nc


def kernel(x, edge_index, W1, b1, W2, b2, Wmu, bmu, Wls, bls):
    global _last_exec_ns
    x = np.asarray(x, np.float32)
    dinv, cores, Wr, idx_pass, idx_comb = _preprocess(edge_index)
    pass_len = [idx_pass[0][r].shape[1] for r in range(4)]

    nc = _build_program(Wr, pass_len)

    def btile(b):
        return np.tile(np.asarray(b, np.float32)[None, :], (128, 1))

    in_maps = []
    for c in range(NC):
        cc = cores[c]
        xT = np.zeros((128, NPAD), np.float32)
        xT[:, cc["rank_of"]] = x[c * NLOC:(c + 1) * NLOC].T

        dv = np.zeros((128, G), np.float32)
        rr = np.arange(128)[:, None] + 128 * np.arange(G)[None, :]
        mreal = rr < NLOC
        dv[mreal] = dinv[c * NLOC + cc["order"][rr[mreal]]]

        im = dict(xT=xT, dinv=dv,
                  dinvx=np.repeat(dv[:, :, None], F3, axis=2),
                  w1=np.asarray(W1, np.float32),
                  w2=np.asarray(W2, np.float32),
                  wmu=np.asarray(Wmu, np.float32),
                  wls=np.asarray(Wls, np.float32),
                  b1t=btile(b1), b2t=btile(b2), bmut=btile(bmu),
                  blst=btile(bls))
        for r in range(4):
            im[f"idxp{r}"] = idx_pass[c][r]
        for p in range(2):
            im[f"idxc{p}"] = idx_comb[c][p]
        in_maps.append(im)

    from concourse.bass_utils import run_bass_kernel_spmd
    res = run_bass_kernel_spmd(nc, in_maps, core_ids=list(range(NC)),
                               trace=_PROFILE, tmpdir=_TMPDIR)
    _last_exec_ns = res.exec_time_ns

    mu = np.empty((N, F4), np.float32)
    ls = np.empty((N, F4), np.float32)
    rr = np.arange(128)[:, None] + 128 * np.arange(G)[None, :]
    mreal = rr < NLOC
    for c in range(NC):
        mo = np.asarray(res.results[c]["mu"]).reshape(128, G, F4)
        lo = np.asarray(res.results[c]["ls"]).reshape(128, G, F4)
        nodes = c * NLOC + cores[c]["order"][rr[mreal]]
        mu[nodes] = mo[mreal]
        ls[nodes] = lo[mreal]
    return mu, ls

